# revision 1
# baseline (speedup 1.0000x reference)
"""Trainium2 Bass kernel: Luong-style attention with source-length masking.

reference math (per batch b):
    keys  = hs @ W_a                      [Ts, H]
    score = ht @ keys^T                   [Tt, Ts]
    e     = exp(score - rowmax)           (masked positions forced to 0)
    a     = e / rowsum(e)
    c     = a @ hs                        [Tt, H]
    out   = tanh(concat([c, ht]) @ W_c + b)

Sharding: batch B=16 data-parallel over 8 NeuronCores (2 batches/core);
W_a / W_c / b replicated. No collectives.

v2 schedule: the two batches per core are software-pipelined so the PE
never sits idle behind DMA or the softmax's vector/scalar chain:
  warmup | T_hs(0) | K(0)+T_ht(0) | S(0) | X(0)v + T_hs(1) | aT(0) | C(0)
  | T_ht(1) | K(1) | S(1) | X(1)v | O(0) (aT(1) transposes slotted in)
  | C(1) | O(1)
W_c's c-half is bf16 (converted on GpSimd so the cast never queues in
front of softmax work); the ht-half of the projection runs in f32r
directly off htT, so no per-batch bf16 copy of ht is needed.  Score
path and PE transposes run in float32r (full-rate fp32).
"""

import numpy as np
from contextlib import ExitStack

import concourse.bass as bass
import concourse.bacc as bacc
import concourse.mybir as mybir
import concourse.tile as tile
from concourse.bass_utils import run_bass_kernel_spmd
from concourse.masks import make_identity

B, TT, TS, H, O = 16, 512, 512, 1024, 1024
NCORES = 8
BL = B // NCORES  # batches per core

F32 = mybir.dt.float32
F32R = mybir.dt.float32r
BF16 = mybir.dt.bfloat16
I32 = mybir.dt.int32

P = 128
KT = H // P    # 8 hidden tiles
NTT = TT // P  # 4 target tiles
NST = TS // P  # 4 source tiles
OCH = 512      # out-projection N chunk (one PSUM bank)
NOC = O // OCH

AX = mybir.AxisListType
ALU = mybir.AluOpType
ACT = mybir.ActivationFunctionType


def build_core(use_bias: bool = True) -> bass.Bass:
    nc = bacc.Bacc()
    ht_d = nc.declare_dram_parameter("ht", [BL, TT, H], F32, isOutput=False)
    hs_d = nc.declare_dram_parameter("hs", [BL, TS, H], F32, isOutput=False)
    src_d = nc.declare_dram_parameter("source", [BL, TS], I32, isOutput=False)
    wa_d = nc.declare_dram_parameter("W_a", [H, H], F32, isOutput=False)
    wc_d = nc.declare_dram_parameter("W_c", [2 * H, O], F32, isOutput=False)
    b_d = nc.declare_dram_parameter("b", [O], F32, isOutput=False)
    out_d = nc.declare_dram_parameter("out", [BL, TT, O], F32, isOutput=True)

    with ExitStack() as ctx:
        tc = ctx.enter_context(tile.TileContext(nc))
        const = ctx.enter_context(tc.tile_pool(name="const", bufs=1))
        wpool = ctx.enter_context(tc.tile_pool(name="weights", bufs=1))
        stage = ctx.enter_context(tc.tile_pool(name="stage", bufs=2))
        natp = ctx.enter_context(tc.tile_pool(name="nat", bufs=2))
        tpose = ctx.enter_context(tc.tile_pool(name="tpose", bufs=1))
        keysp = ctx.enter_context(tc.tile_pool(name="keysp", bufs=3))
        nath = ctx.enter_context(tc.tile_pool(name="nath", bufs=4))
        htp = ctx.enter_context(tc.tile_pool(name="htp", bufs=2))
        bfp = ctx.enter_context(tc.tile_pool(name="bf", bufs=1))
        htbfp = ctx.enter_context(tc.tile_pool(name="htbf", bufs=2))
        onep = ctx.enter_context(tc.tile_pool(name="one", bufs=1))
        abfp = ctx.enter_context(tc.tile_pool(name="abf", bufs=4))
        outp = ctx.enter_context(tc.tile_pool(name="outs", bufs=1))
        maskp = ctx.enter_context(tc.tile_pool(name="maskrow", bufs=1))
        penp = ctx.enter_context(tc.tile_pool(name="pen", bufs=1))
        stats = ctx.enter_context(tc.tile_pool(name="stats", bufs=4))
        pmm = ctx.enter_context(tc.tile_pool(name="pmm", bufs=2, space="PSUM"))
        ptr = ctx.enter_context(tc.tile_pool(name="ptr", bufs=2, space="PSUM"))
        psc = ctx.enter_context(tc.tile_pool(name="psc", bufs=4, space="PSUM"))

        # ---------------- constants ----------------
        ident_f = stage.tile([P, P], F32, name="wcstage")
        make_identity(nc, ident_f[:])
        ident_r = const.tile([P, P], F32R)
        nc.vector.tensor_copy(ident_r[:], ident_f[:])
        ident_bf = const.tile([P, P], BF16)
        make_identity(nc, ident_bf[:])
        # PE warm-up: throwaway transposes release the HAM clock-gate while
        # the first input DMAs land.
        wtile = pmm.tile([P, TS], F32R, name="mm_ps")
        for _ in range(16):
            nc.tensor.transpose(wtile[:, 0:P], ident_r[:], ident_r[:])

        ones_f32 = stage.tile([1, P], F32, name="wcstage")
        nc.vector.memset(ones_f32[:], 1.0)
        ones_f = const.tile([1, P], F32R)
        nc.vector.tensor_copy(ones_f[:], ones_f32[:])

        iota_f = const.tile([1, TS], F32)
        nc.gpsimd.iota(
            iota_f[:],
            pattern=[[1, TS]],
            base=0,
            channel_multiplier=0,
            allow_small_or_imprecise_dtypes=True,
        )

        # ---------------- weights (GpSimd queue only) ----------------
        # W_a row-tiled: contiguous 4KB rows DMA at full rate (the
        # column-sliced variant reads 512B strided rows and paces keysT
        # ~5us/slice).  keysT starts once all rows land (~14us).
        wa_sb = wpool.tile([P, KT, H], F32R)  # [k in kt, kt, l]
        for kh in range(KT // 2):
            nc.gpsimd.dma_start(
                out=wa_sb[:, 2 * kh : 2 * kh + 2, :],
                in_=wa_d[2 * kh * P : (2 * kh + 2) * P, :]
                .rearrange("(kt p) l -> p kt l", p=P)
                .bitcast(F32R),
            )
        # bias row straight to f32r; the bias matmul runs f32r w/ ones_f
        b_r = None
        if use_bias:
            b_r = stage.tile([1, O], F32R, name="bstage")
            nc.gpsimd.dma_start(
                out=b_r[:], in_=b_d.rearrange("(a o) -> a o", a=1).bitcast(F32R)
            )
        # all of W_c in bf16 (f32r weights forgo fast-weight-load, which
        # costs ~400ns per out group): DMAs through a 2-slot staging
        # ring, bf16 casts on ScalarE mid-schedule (a cast on GpSimd
        # costs 3.5us and serializes the SWDGE queue).  [:, 0:KT] is the
        # c-half, [:, KT:] the ht-half.
        wc_bf = wpool.tile([P, 2 * KT, O], BF16)
        wc_stages = []
        for kt in range(2 * KT):
            wst = stage.tile([P, O], F32, name="wcstage")
            nc.gpsimd.dma_start(out=wst[:], in_=wc_d[kt * P : (kt + 1) * P, :])
            wc_stages.append(wst)

        def wc_cast_block():
            # interleave c-half/ht-half casts so both halves are ready
            # well before the first out group consumes them
            for kt in range(KT):
                nc.scalar.copy(wc_bf[:, kt, :], wc_stages[kt][:])
                nc.scalar.copy(wc_bf[:, KT + kt, :], wc_stages[KT + kt][:])

        # ---------------- per-batch tile handles ----------------
        hsT = tpose.tile([P, KT, TS], F32R, name="hsT")      # [k, kt, s]
        htT = {}
        htT_bf = {}
        hs_bf = {}
        pen_row = {}

        # ---------------- phase emitters ----------------
        def mask_prep(bi):
            src_sb = maskp.tile([1, TS], I32, name="src")
            nc.sync.dma_start(out=src_sb[:], in_=src_d[bi : bi + 1, :])
            # not_equal in place over the src tile (bitcast view as F32 out)
            pr_f = src_sb[:].bitcast(F32)
            nc.vector.tensor_scalar(pr_f, src_sb[:], 0, None, ALU.not_equal)
            lens = stats.tile([1, 1], F32, name="lens")
            nc.vector.reduce_sum(out=lens[:], in_=pr_f, axis=AX.X)
            # (iota >= len) * -1e9 : -1e9 at masked positions, 0 at valid
            nc.vector.tensor_scalar(
                pr_f, iota_f[:], lens[:], -1e9, ALU.is_ge, ALU.mult
            )
            pr = penp.tile([1, TS], F32R, name="pen_row")
            nc.vector.tensor_copy(pr[:], pr_f[:])
            pen_row[bi] = pr

        def t_hs(bi, sts=None):
            """Load hs, cast to bf16, transpose into hsT (f32r)."""
            if bi not in hs_bf:
                hs_bf[bi] = bfp.tile([P, NST, H], BF16, name="hs_bf")
            hb = hs_bf[bi]
            for st in sts if sts is not None else range(NST):
                nat = natp.tile([P, H], F32R, name="nat")
                nc.sync.dma_start(
                    out=nat[:], in_=hs_d[bi, st * P : (st + 1) * P, :].bitcast(F32R)
                )
                nc.scalar.copy(hb[:, st, :], nat[:])
                for kh in range(2):
                    tp4 = ptr.tile([P, 4, P], F32R, name="tp")
                    for kj in range(4):
                        kt = kh * 4 + kj
                        nc.tensor.transpose(
                            tp4[:, kj, :], nat[:, kt * P : (kt + 1) * P], ident_r[:]
                        )
                    nc.vector.tensor_copy(
                        hsT[:, kh * 4 : (kh + 1) * 4, st * P : (st + 1) * P], tp4[:]
                    )

        def t_ht_load(bi):
            # ht rides the ScalarE HWDGE ring so it streams in parallel
            # with hs on the Sync ring (~210 GB/s per ring).
            nats = []
            for tt in range(NTT):
                nat = nath.tile([P, H], F32R, name="ht_nat")
                nc.scalar.dma_start(
                    out=nat[:], in_=ht_d[bi, tt * P : (tt + 1) * P, :].bitcast(F32R)
                )
                nats.append(nat)
            return nats

        def t_ht_tile(bi, tt, nat):
            """Transpose one ht tile into htT[bi]."""
            for kh in range(2):
                tp4 = ptr.tile([P, 4, P], F32R, name="tp")
                for kj in range(4):
                    kt = kh * 4 + kj
                    nc.tensor.transpose(
                        tp4[:, kj, :], nat[:, kt * P : (kt + 1) * P], ident_r[:]
                    )
                nc.vector.tensor_copy(
                    htT[bi][:, kh * 4 : (kh + 1) * 4, tt * P : (tt + 1) * P], tp4[:]
                )
            nc.scalar.copy(
                htT_bf[bi][:, :, tt * P : (tt + 1) * P],
                htT[bi][:, :, tt * P : (tt + 1) * P],
            )

        def keys_scores(bi):
            """keysT and score fused: each keysT slice is consumed by the
            4 score partials one step behind (software pipelined), so the
            full [P,KT,TS] keysT tensor never materializes."""
            sc_ps = [psc.tile([P, TS], F32, name="sc_ps") for _ in range(NTT)]
            ks = {}

            def k_group(lt):
                ps = pmm.tile([P, TS], F32, name="mm_ps")
                for kt in range(KT):
                    nc.tensor.matmul(
                        ps[:],
                        lhsT=wa_sb[:, kt, lt * P : (lt + 1) * P],
                        rhs=hsT[:, kt, :],
                        start=(kt == 0),
                        stop=(kt == KT - 1),
                    )
                sl = keysp.tile([P, TS], F32R, name="keys_sl")
                nc.vector.tensor_copy(sl[:], ps[:])
                ks[lt] = sl

            def s_partial(lt):
                for tt in range(NTT):
                    nc.tensor.matmul(
                        sc_ps[tt][:],
                        lhsT=htT[bi][:, lt, tt * P : (tt + 1) * P],
                        rhs=ks[lt][:],
                        start=(lt == 0),
                        stop=False,
                    )

            k_group(0)
            for lt in range(1, KT):
                k_group(lt)
                s_partial(lt - 1)
            s_partial(KT - 1)
            for tt in range(NTT):
                # fold the mask penalty in as a K=1 broadcast accumulation
                nc.tensor.matmul(
                    sc_ps[tt][:],
                    lhsT=ones_f[:],
                    rhs=pen_row[bi][:],
                    start=False,
                    stop=True,
                )
            return sc_ps

        def softmax_chains(sc_ps):
            """Vector/Scalar-only part of the masked softmax."""
            negms = []
            for tt in range(NTT):
                negm = stats.tile([P, 1], F32, name="negm")
                nc.vector.reduce_max(
                    out=negm[:], in_=sc_ps[tt][:], axis=AX.X, negate=True
                )
                negms.append(negm)
            abfs = []
            for tt in range(NTT):
                d = stats.tile([P, 1], F32, name="d")
                # exp in place in the score PSUM bank (saves an SBUF tile)
                nc.scalar.activation(
                    out=sc_ps[tt][:], in_=sc_ps[tt][:], func=ACT.Exp,
                    bias=negms[tt][:], scale=1.0, accum_out=d[:],
                )
                dr = stats.tile([P, 1], F32, name="dr")
                nc.vector.reciprocal(dr[:], d[:])
                abf = abfp.tile([P, TS], BF16, name="abf")
                nc.vector.tensor_scalar(abf[:], sc_ps[tt][:], dr[:], None, ALU.mult)
                abfs.append(abf)
            return abfs

        def a_transpose(abfs, aT=None, tts=None):
            if aT is None:
                aT = onep.tile([P, NST, TT], BF16, name="aT")
            for tt in tts if tts is not None else range(NTT):
                tpb = ptr.tile([P, 4, P], BF16, name="tp")
                for st in range(NST):
                    nc.tensor.transpose(
                        tpb[:, st, :], abfs[tt][:, st * P : (st + 1) * P], ident_bf[:]
                    )
                nc.vector.tensor_copy(aT[:, :, tt * P : (tt + 1) * P], tpb[:])
            return aT

        def ctx_mm(bi, aT):
            cT_bf = onep.tile([P, KT, TT], BF16, name="cT")
            for kt in range(KT):
                c_ps = pmm.tile([P, TT], F32, name="mm_ps")
                for st in range(NST):
                    nc.tensor.matmul(
                        c_ps[:],
                        lhsT=hs_bf[bi][:, st, kt * P : (kt + 1) * P],
                        rhs=aT[:, st, :],
                        start=(st == 0),
                        stop=(st == NST - 1),
                    )
                nc.vector.tensor_copy(cT_bf[:, kt, :], c_ps[:])
            return cT_bf

        def out_group(bi, cT_bf, tt, oc):
            # rotate through the 4 score banks (idle during the out
            # phases) so a group start never waits on a tanh drain
            o_ps = psc.tile([P, OCH], F32, name="sc_ps")
            for kt in range(KT):
                nc.tensor.matmul(
                    o_ps[:],
                    lhsT=cT_bf[:, kt, tt * P : (tt + 1) * P],
                    rhs=wc_bf[:, kt, oc * OCH : (oc + 1) * OCH],
                    start=(kt == 0),
                    stop=False,
                )
            for kt in range(KT):
                nc.tensor.matmul(
                    o_ps[:],
                    lhsT=htT_bf[bi][:, kt, tt * P : (tt + 1) * P],
                    rhs=wc_bf[:, KT + kt, oc * OCH : (oc + 1) * OCH],
                    start=False,
                    stop=(not use_bias and kt == KT - 1),
                )
            if use_bias:
                nc.tensor.matmul(
                    o_ps[:],
                    lhsT=ones_f[:],
                    rhs=b_r[:, oc * OCH : (oc + 1) * OCH],
                    start=False,
                    stop=True,
                )
            ot = outp.tile([P, OCH], F32, name="out_t")
            nc.scalar.activation(out=ot[:], in_=o_ps[:], func=ACT.Tanh)
            nc.sync.dma_start(
                out=out_d[bi, tt * P : (tt + 1) * P, oc * OCH : (oc + 1) * OCH],
                in_=ot[:],
            )

        # ---------------- pipelined schedule over the 2 batches ----------
        mask_prep(0)
        htT[0] = htp.tile([P, KT, TT], F32R, name="htT")
        htT[1] = htp.tile([P, KT, TT], F32R, name="htT")
        htT_bf[0] = htbfp.tile([P, KT, TT], BF16, name="htT_bf")
        htT_bf[1] = htbfp.tile([P, KT, TT], BF16, name="htT_bf")

        # ht DMAs ride the scalar ring; b1's follow b0's hs_bf copies
        # in the scalar stream (issuing them earlier deadlocks the nat
        # staging rings against the PE transpose order)
        ht0_nats = t_ht_load(0)

        # b0: input loads + transposes fill the PE until W_a lands
        t_hs(0)
        ht1_nats = t_ht_load(1)
        for tt in range(NTT):
            t_ht_tile(0, tt, ht0_nats[tt])
        sc0 = keys_scores(0)

        # b0 softmax -> aT -> C before b1's hs staging: hs_bf has a
        # single slot, so C(0) must drain before b1's copies can land
        mask_prep(1)
        abfs0 = softmax_chains(sc0)
        wc_cast_block()
        # b1's ht transposes fill the PE while b0's softmax chain runs
        for tt in range(NTT):
            t_ht_tile(1, tt, ht1_nats[tt])
        aT0 = a_transpose(abfs0)
        cT0 = ctx_mm(0, aT0)
        t_hs(1)
        sc1 = keys_scores(1)

        # b1 softmax (vector/scalar) overlaps b0's out projection
        abfs1 = softmax_chains(sc1)
        og = [(tt, oc) for tt in range(NTT) for oc in range(NOC)]
        for tt, oc in og[:3]:
            out_group(0, cT0, tt, oc)
        aT1 = a_transpose(abfs1)
        for tt, oc in og[3:]:
            out_group(0, cT0, tt, oc)
        cT1 = ctx_mm(1, aT1)
        for tt, oc in og:
            out_group(1, cT1, tt, oc)

    return nc


def make_in_maps(ht, hs, source, W_a, W_c, b):
    ht = np.ascontiguousarray(ht, dtype=np.float32)
    hs = np.ascontiguousarray(hs, dtype=np.float32)
    source = np.ascontiguousarray(source, dtype=np.int32)
    W_a = np.ascontiguousarray(W_a, dtype=np.float32)
    W_c = np.ascontiguousarray(W_c, dtype=np.float32)
    b = np.ascontiguousarray(b, dtype=np.float32)
    in_maps = []
    for c in range(NCORES):
        sl = slice(c * BL, (c + 1) * BL)
        in_maps.append(
            {
                "ht": ht[sl],
                "hs": hs[sl],
                "source": source[sl],
                "W_a": W_a,
                "W_c": W_c,
                "b": b,
            }
        )
    return in_maps


_NC_CACHE: dict = {}


def _get_nc(use_bias: bool = True):
    key = f"nc_bias{use_bias}"
    if key not in _NC_CACHE:
        nc = build_core(use_bias=use_bias)
        if not nc.is_finalized():
            nc.finalize()
        _NC_CACHE[key] = nc
    return _NC_CACHE[key]


def run_on_hw(ht, hs, source, W_a, W_c, b, trace=False, **kw):
    nc = _get_nc(use_bias=bool(np.any(np.asarray(b) != 0)))
    in_maps = make_in_maps(ht, hs, source, W_a, W_c, b)
    res = run_bass_kernel_spmd(nc, in_maps, core_ids=list(range(NCORES)), trace=trace, **kw)
    out = np.concatenate([res.results[c]["out"] for c in range(NCORES)], axis=0)
    return out, res


def kernel(ht, hs, source, W_a, W_c, b):
    out, _ = run_on_hw(ht, hs, source, W_a, W_c, b, trace=False)
    return out

